# revision 13
# baseline (speedup 1.0000x reference)
"""MixerDiffAttention Trainium2 kernel, v3.

Sharding: 8 cores = batch(2) x head-group(4).  Core (b, r) computes output
heads {2r, 2r+1} of batch b.

v3 over v2 (374us):
 - fp16 x / weights (host-preconverted) and fp16 q/k after rope.  fp16's
   10-bit mantissa keeps the final rel err at ~6e-3 (bf16 would be 2.5e-2).
   No f32r anywhere -> no casting DMAs -> loads can use all 3 DMA queues
   (gpsimd/sync/scalar), LDWEIGHTS drops 188->97ns, transposes 1.5->1.0
   cycles/row, fp32_mode=HIGH power draw is gone, and DMA bytes halve.
 - ALL of x (8MB fp16 = 64 tiles) preloaded at startup across the 3 queues,
   (x-chunk0, wq, wkv) trios rotated first so dmi-d inputs land by ~0.5*d us.
   No mid-kernel x DMA gating at all (v2 stalled ~4us x4 on late x chunks).
 - diagonal score matmuls trimmed to the unmasked column range.
 - ps_p 3 banks so proj of tt+1 isn't gated on tt's rope PSUM reads.
 - eager output normalization for both head blocks of the last chunk.
"""
import numpy as np
import concourse.bass as bass
import concourse.mybir as mybir
from concourse import bacc
from concourse.tile import TileContext
from concourse.bass_utils import run_bass_kernel_spmd

B, T, DM = 2, 2048, 2048
H, KVH, D = 16, 8, 128
TC = 512                  # token chunk (= q chunk)
NT = T // 128             # 16 token tiles
NCH = T // TC             # 4 chunks
NDM = DM // 128           # 16 contraction chunks
EPS = 1e-6
ROPE_BASE = 10000.0
LAMBDA_INIT = 0.8 - 0.6 * np.exp(-0.3 * 12)
F32 = mybir.dt.float32
BF16 = mybir.dt.bfloat16
FP16 = mybir.dt.float16
AF = mybir.ActivationFunctionType
ALU = mybir.AluOpType
AX = mybir.AxisListType
ISQ = float(1.0 / np.sqrt(D))


def _bc_mid(a, n):
    # [128, m] AP -> [128, n(bcast), m]
    return bass.AP(tensor=a.tensor, offset=a.offset, ap=[list(a.ap[0]), [0, n], list(a.ap[1])])


def _bc_last(a, n):
    # [128, m] AP -> [128, m, n(bcast)]
    return bass.AP(tensor=a.tensor, offset=a.offset, ap=[list(a.ap[0]), list(a.ap[1]), [0, n]])


class _Bacc(bacc.Bacc):
    """Bacc that pins every activation to the natural_log_exp_and_others
    table set (it contains exp, ln, square, copy - all this kernel uses),
    so exactly one ACT_TABLE_LOAD is emitted instead of one per ln<->exp
    alternation."""

    def insert_act_table_loads(self):
        import bass_rust as _bass_rust
        from concourse.hw_specs import get_activation_tables

        has_activation = any(
            isinstance(i, mybir.InstActivation)
            for b in self.main_func.blocks
            for i in b.instructions
        )
        if not has_activation:
            return
        keep = "natural_log_exp_and_others"
        tables = [(n, (s if n == keep else set()))
                  for n, s in get_activation_tables(self.m.arch).items()]
        _bass_rust.insert_act_table_loads(self, tables)


def _build():
    nc = _Bacc(None, target_bir_lowering=False)

    xT = nc.dram_tensor("xT", [DM, T], FP16, kind="ExternalInput")
    wq_d = nc.dram_tensor("wq", [DM, 512], FP16, kind="ExternalInput")
    wkv_d = nc.dram_tensor("wkv", [DM, 512], FP16, kind="ExternalInput")
    lsc_d = nc.dram_tensor("lsc", [128, NT, 4], F32, kind="ExternalInput")
    gn_d = nc.dram_tensor("gnw", [128, 2, 256], F32, kind="ExternalInput")
    neglam_d = nc.dram_tensor("neglam", [128, 1], F32, kind="ExternalInput")
    y_d = nc.dram_tensor("y", [T, 2, 256], F32, kind="ExternalOutput")

    pos = np.arange(T, dtype=np.float64)
    inv = ROPE_BASE ** (-np.arange(0, D, 2, dtype=np.float64) / D)  # (64,)
    ang = np.outer(pos, inv)
    cos_h = np.cos(ang).astype(np.float32).reshape(NT, 128, 64).transpose(1, 0, 2).copy()
    sin_h = np.sin(ang).astype(np.float32).reshape(NT, 128, 64).transpose(1, 0, 2).copy()
    pidx = np.arange(128).reshape(128, 1)
    qidx = np.arange(128).reshape(1, 128)
    tri_h = (pidx <= qidx).astype(np.float32)        # keep k<=q in diag block
    ident_h = np.eye(128, dtype=np.float16)

    cos_c = nc.inline_tensor(cos_h, "cos_c")
    sin_c = nc.inline_tensor(sin_h, "sin_c")
    tri_c = nc.inline_tensor(tri_h, "tri_c")
    ident_c = nc.inline_tensor(ident_h, "ident_c")

    with TileContext(nc) as tc:
        with (
            tc.tile_pool(name="wp", bufs=1) as wp,
            tc.tile_pool(name="cp", bufs=1) as cp,
            tc.tile_pool(name="xp", bufs=1) as xp,
            tc.tile_pool(name="kv", bufs=1) as kvp,
            tc.tile_pool(name="qt", bufs=2) as qtp,
            tc.tile_pool(name="wk", bufs=2) as wk,
            tc.tile_pool(name="yo", bufs=1) as yop,
            tc.tile_pool(name="yv", bufs=5) as yvp,
            tc.tile_pool(name="ps_p", bufs=3, space="PSUM") as ps_p,
            tc.tile_pool(name="ps_t", bufs=1, space="PSUM") as ps_t,
            tc.tile_pool(name="ps_x", bufs=4, space="PSUM") as ps_x,
        ):
            # ---- persistent tiles ----
            # per-dmi weight tiles: the dmi-0 projection matmul only waits on
            # its own DMA, not all 16.
            wq_t = [wp.tile([128, 512], FP16, tag=f"wq{i}", name=f"wq{i}")
                    for i in range(NDM)]
            wkv_t = [wp.tile([128, 512], FP16, tag=f"wkv{i}", name=f"wkv{i}")
                     for i in range(NDM)]
            # all 64 x tiles resident (fp16); [dmi][chunk]
            x_t = [[xp.tile([128, TC], FP16, tag=f"x{d}_{c}", name=f"x{d}_{c}")
                    for c in range(NCH)] for d in range(NDM)]

            cos_sb = cp.tile([128, NT, 64], F32, tag="cos")
            sin_sb = cp.tile([128, NT, 64], F32, tag="sin")
            tri_sb = cp.tile([128, 128], BF16, tag="tri")
            ident_sb = cp.tile([128, 128], FP16, tag="ident")
            lsc_sb = cp.tile([128, NT, 4], F32, tag="lsc")
            gn_sb = cp.tile([128, 2, 256], F32, tag="gn")
            neglam_sb = cp.tile([128, 1], F32, tag="neglam")
            eps_sb = cp.tile([128, 1], F32, tag="eps")
            lnisq_sb = cp.tile([128, 1], F32, tag="lnisq")
            nc.sync.dma_start(out=cos_sb, in_=cos_c.ap())
            nc.sync.dma_start(out=sin_sb, in_=sin_c.ap())
            nc.gpsimd.dma_start(out=tri_sb, in_=tri_c.ap())
            nc.gpsimd.dma_start(out=ident_sb, in_=ident_c.ap())
            nc.sync.dma_start(out=lsc_sb, in_=lsc_d.ap())
            nc.sync.dma_start(out=gn_sb, in_=gn_d.ap())
            nc.sync.dma_start(out=neglam_sb, in_=neglam_d.ap())
            nc.vector.memset(eps_sb[:], EPS)
            nc.vector.memset(lnisq_sb[:], float(np.log(ISQ)))

            # persistent per-token-tile K^T (fp16) and V+ones (bf16)
            kT_t = [kvp.tile([128, 2, 128], FP16, tag=f"kT{i}", name=f"kT{i}") for i in range(NT)]
            vA_t = [kvp.tile([128, 258], BF16, tag=f"vA{i}", name=f"vA{i}") for i in range(NT)]
            # probability store [128, kt, s, 512] bf16 (bf16 for range: exp is
            # taken without max-subtraction); zeroed once (below, on gpsimd).
            pt_sb = kvp.tile([128, NT, 2, 512], BF16, tag="pt")
            rsk_sb = kvp.tile([128, NT, 2], F32, tag="rsk")
            ssy_sb = kvp.tile([128, 8], F32, tag="ssy")
            rsy_sb = kvp.tile([128, 8], F32, tag="rsy")

            def proj_gen(c):
                """generator: projection + q/k processing for chunk c, in
                pieces (a yield ~= 4 PE matmuls).  All q matmuls go before all
                kv matmuls: the kv PSUM allocation then doesn't make the
                in-order PE queue wait for the previous tile's rope reads."""
                qT_a = qtp.tile([128, 4, 256], FP16, tag="qta")
                qT_b = qtp.tile([128, 4, 256], FP16, tag="qtb")
                qT_all[c] = (qT_a, qT_b)
                for ti in range(4):
                    tt = c * 4 + ti
                    q_ps = ps_p.tile([128, 512], F32, tag="pp")
                    kv_ps = ps_p.tile([128, 512], F32, tag="pp")
                    for dmi in range(NDM):
                        lhs = x_t[dmi][c][:, ti * 128:(ti + 1) * 128]
                        nc.tensor.matmul(q_ps[:], lhs, wq_t[dmi][:],
                                         start=(dmi == 0), stop=(dmi == NDM - 1))
                        if dmi % 2 == 1:
                            yield
                    for dmi in range(NDM):
                        lhs = x_t[dmi][c][:, ti * 128:(ti + 1) * 128]
                        nc.tensor.matmul(kv_ps[:], lhs, wkv_t[dmi][:],
                                         start=(dmi == 0), stop=(dmi == NDM - 1))
                        if dmi % 2 == 1:
                            yield

                    # rms stats: scalar square (f32 scratch) -> vector reduce
                    sqq = wk.tile([128, 512], F32, tag="sqq")
                    sqk = wk.tile([128, 256], F32, tag="sqk")
                    nc.scalar.activation(out=sqq[:], in_=q_ps[:], func=AF.Square)
                    nc.scalar.activation(out=sqk[:], in_=kv_ps[:, 0:256], func=AF.Square)
                    ssq = wk.tile([128, 4], F32, tag="ssq")
                    ssk = wk.tile([128, 2], F32, tag="ssk")
                    nc.vector.reduce_sum(ssq[:], sqq[:].rearrange("p (h d) -> p h d", h=4), axis=AX.X)
                    nc.vector.reduce_sum(ssk[:], sqk[:].rearrange("p (h d) -> p h d", h=2), axis=AX.X)
                    lq = wk.tile([128, 4], F32, tag="lq")
                    nc.scalar.activation(out=lq[:], in_=ssq[:], func=AF.Ln,
                                         scale=1.0 / D, bias=eps_sb[:])
                    rsq0 = wk.tile([128, 4], F32, tag="rsq0")
                    nc.scalar.activation(out=rsq0[:], in_=lq[:], func=AF.Exp, scale=-0.5)
                    lk = wk.tile([128, 2], F32, tag="lk")
                    nc.scalar.activation(out=lk[:], in_=ssk[:], func=AF.Ln,
                                         scale=1.0 / D, bias=eps_sb[:])
                    nc.scalar.activation(out=rsk_sb[:, tt, :], in_=lk[:], func=AF.Exp,
                                         scale=-0.5, bias=lnisq_sb[:])
                    # rope reads proj PSUM directly, writes fp16; scale applied
                    # after (rotation commutes with the per-(token,head) scale).
                    # All four PSUM reads go first so the bank frees early.
                    qr = wk.tile([128, 4, 128], FP16, tag="qr")
                    qv = q_ps[:].rearrange("p (h d) -> p h d", h=4)
                    cos4 = _bc_mid(cos_sb[:, tt, :], 4)
                    sin4 = _bc_mid(sin_sb[:, tt, :], 4)
                    t1 = wk.tile([128, 4, 64], F32, tag="t1")
                    t1b = wk.tile([128, 4, 64], F32, tag="t1b")
                    nc.vector.tensor_mul(t1[:], qv[:, :, 64:128], sin4)
                    nc.vector.tensor_mul(t1b[:], qv[:, :, 0:64], sin4)
                    nc.vector.tensor_mul(qr[:, :, 0:64], qv[:, :, 0:64], cos4)
                    nc.vector.tensor_mul(qr[:, :, 64:128], qv[:, :, 64:128], cos4)
                    nc.vector.tensor_add(qr[:, :, 0:64], qr[:, :, 0:64], t1[:])
                    nc.vector.tensor_sub(qr[:, :, 64:128], qr[:, :, 64:128], t1b[:])
                    rsq = wk.tile([128, 4], F32, tag="rsq")
                    nc.vector.tensor_mul(rsq[:], rsq0[:], lsc_sb[:, tt, :])
                    nc.vector.tensor_mul(qr[:], qr[:], _bc_last(rsq[:], 128))
                    yield

                    # v copy (vector); k rope reads PSUM directly, writes fp16
                    nc.vector.tensor_copy(out=vA_t[tt][:, 0:256], in_=kv_ps[:, 256:512])
                    kr = wk.tile([128, 2, 128], FP16, tag="kr")
                    kv_ = kv_ps[:, 0:256].rearrange("p (h d) -> p h d", h=2)
                    cos2 = _bc_mid(cos_sb[:, tt, :], 2)
                    sin2 = _bc_mid(sin_sb[:, tt, :], 2)
                    t2 = t1[:, 0:2, :]
                    t2b = t1b[:, 0:2, :]
                    nc.vector.tensor_mul(t2, kv_[:, :, 64:128], sin2)
                    nc.vector.tensor_mul(t2b, kv_[:, :, 0:64], sin2)
                    nc.vector.tensor_mul(kr[:, :, 0:64], kv_[:, :, 0:64], cos2)
                    nc.vector.tensor_mul(kr[:, :, 64:128], kv_[:, :, 64:128], cos2)
                    nc.vector.tensor_add(kr[:, :, 0:64], kr[:, :, 0:64], t2)
                    nc.vector.tensor_sub(kr[:, :, 64:128], kr[:, :, 64:128], t2b)
                    yield

                    # transposes (PE, fp16, all 6 into one PSUM bank) +
                    # batched PSUM->SBUF copies (vector)
                    tqk = ps_t.tile([128, 768], FP16, tag="tqk")
                    qrf = qr[:].rearrange("p h d -> p (h d)")
                    for h in range(4):
                        nc.tensor.transpose(tqk[:, h * 128:(h + 1) * 128],
                                            qrf[:, h * 128:(h + 1) * 128], ident_sb[:])
                    krf = kr[:].rearrange("p h d -> p (h d)")
                    for h in range(2):
                        nc.tensor.transpose(tqk[:, (4 + h) * 128:(5 + h) * 128],
                                            krf[:, h * 128:(h + 1) * 128], ident_sb[:])
                    qdst = qT_a if ti < 2 else qT_b
                    to = (ti % 2) * 128
                    nc.vector.tensor_copy(
                        out=qdst[:][:, :, to:to + 128],
                        in_=tqk[:, 0:512].rearrange("p (h t) -> p h t", h=4))
                    nc.vector.tensor_copy(out=kT_t[tt][:], in_=tqk[:, 512:768].rearrange("p (h t) -> p h t", h=2))
                    yield

            qT_all = {}

            def pv_out_block(c, h, pace=None, sq_desc=False, kt_desc=False,
                             eager_rs=False):
                """PV accumulation + output combine for chunk c, head h."""
                yv_tiles = {}
                sqs = (3, 2, 1, 0) if sq_desc else (0, 1, 2, 3)
                for sq in sqs:
                    qt_g = 4 * c + sq
                    o0 = ps_x.tile([128, 258], F32, tag="st")
                    o1 = ps_x.tile([128, 258], F32, tag="st")
                    for s, o in ((0, o0), (1, o1)):
                        nkt = qt_g + 1
                        kts = range(nkt - 1, -1, -1) if kt_desc else range(nkt)
                        for i, kt in enumerate(kts):
                            nc.tensor.matmul(
                                o[:], pt_sb[:, kt, s, sq * 128:(sq + 1) * 128],
                                vA_t[kt][:], start=(i == 0), stop=(i == nkt - 1))
                        if pace is not None:
                            pace()
                    r0 = yop.tile([128, 1], F32, tag="r0")
                    r1 = yop.tile([128, 1], F32, tag="r1")
                    nc.vector.reciprocal(r0[:], o0[:, 256:257])
                    nc.vector.reciprocal(r1[:], o1[:, 256:257])
                    nc.vector.tensor_mul(r1[:], r1[:], neglam_sb[:])
                    tsb = yop.tile([128, 256], F32, tag="t")
                    nc.vector.tensor_scalar_mul(tsb[:], o1[:, 0:256], r1[:])
                    yv = yvp.tile([128, 256], F32, tag="yv")
                    yv_tiles[sq] = yv
                    nc.vector.scalar_tensor_tensor(
                        out=yv[:], in0=o0[:, 0:256], scalar=r0[:], in1=tsb[:],
                        op0=ALU.mult, op1=ALU.add)
                    nc.scalar.activation(out=tsb[:], in_=yv[:], func=AF.Square)
                    nc.vector.reduce_sum(ssy_sb[:, 4 * h + sq:4 * h + sq + 1],
                                         tsb[:], axis=AX.X)
                    if eager_rs:
                        i0 = 4 * h + sq
                        ly1 = yop.tile([128, 1], F32, tag="ly1")
                        nc.scalar.activation(out=ly1[:], in_=ssy_sb[:, i0:i0 + 1],
                                             func=AF.Ln, scale=1.0 / 256, bias=eps_sb[:])
                        nc.scalar.activation(out=rsy_sb[:, i0:i0 + 1], in_=ly1[:],
                                             func=AF.Exp, scale=-0.5)
                        yo = yop.tile([128, 256], F32, tag="yo")
                        nc.vector.scalar_tensor_tensor(
                            out=yo[:], in0=yv[:], scalar=rsy_sb[:, i0:i0 + 1],
                            in1=gn_sb[:, h, :], op0=ALU.mult, op1=ALU.mult)
                        (nc.sync if sq % 2 == 0 else nc.gpsimd).dma_start(
                            out=y_d.ap()[qt_g * 128:(qt_g + 1) * 128, h, :],
                            in_=yo[:])
                if eager_rs:
                    return
                ly = yop.tile([128, 4], F32, tag="ly")
                nc.scalar.activation(out=ly[:], in_=ssy_sb[:, 4 * h:4 * h + 4],
                                     func=AF.Ln, scale=1.0 / 256, bias=eps_sb[:])
                nc.scalar.activation(out=rsy_sb[:, 4 * h:4 * h + 4], in_=ly[:],
                                     func=AF.Exp, scale=-0.5)
                for sq in range(4):
                    qt_g = 4 * c + sq
                    yo = yop.tile([128, 256], F32, tag="yo")
                    nc.vector.scalar_tensor_tensor(
                        out=yo[:], in0=yv_tiles[sq][:],
                        scalar=rsy_sb[:, 4 * h + sq:4 * h + sq + 1],
                        in1=gn_sb[:, h, :], op0=ALU.mult, op1=ALU.mult)
                    (nc.sync if sq % 2 == 0 else nc.gpsimd).dma_start(
                        out=y_d.ap()[qt_g * 128:(qt_g + 1) * 128, h, :],
                        in_=yo[:])

            def emit_scores(c, h, pace, kt_desc=False, split_exp=False):
                """score matmul + exp for (chunk c, head h), interleaving
                filler pieces (projection of c+1) between steps.  Each score
                matmul is split at column 256 (the qT_a/qT_b tile boundary) so
                the first half only depends on the chunk's first two token
                tiles."""
                qT_a, qT_b = qT_all[c]
                kts = (range(4 * (c + 1) - 1, -1, -1) if kt_desc
                       else range(4 * (c + 1)))
                steps = [(kt, s) for kt in kts for s in range(2)]
                for (kt, s) in steps:
                    j = kt - 4 * c
                    L = max(j, 0) * 128
                    st = ps_x.tile([128, 512], F32, tag="st")
                    if L < 256:
                        nc.tensor.matmul(st[:, L:256], kT_t[kt][:, s, :],
                                         qT_a[:, 2 * s + h, L:256],
                                         start=True, stop=True)
                    nc.tensor.matmul(st[:, max(L, 256):512], kT_t[kt][:, s, :],
                                     qT_b[:, 2 * s + h, max(L, 256) - 256:256],
                                     start=True, stop=True)
                    if split_exp and L < 256:
                        nc.scalar.activation(
                            out=pt_sb[:, kt, s, L:256], in_=st[:, L:256],
                            func=AF.Exp, scale=rsk_sb[:, kt, s:s + 1])
                        nc.scalar.activation(
                            out=pt_sb[:, kt, s, 256:512], in_=st[:, 256:512],
                            func=AF.Exp, scale=rsk_sb[:, kt, s:s + 1])
                    else:
                        nc.scalar.activation(
                            out=pt_sb[:, kt, s, L:512], in_=st[:, L:512],
                            func=AF.Exp, scale=rsk_sb[:, kt, s:s + 1])
                    if j >= 0:
                        nc.vector.tensor_mul(
                            pt_sb[:, kt, s, L:L + 128],
                            pt_sb[:, kt, s, L:L + 128], tri_sb[:])
                    pace()

            # ---------------- main schedule ----------------
            N_PIECES = 4 * (8 + 8 + 3)
            # startup DMAs: rotate the (x-chunk0, wq, wkv) trio across the 3
            # DMA queues per dmi (so dmi-d's three inputs land by ~0.5*d us),
            # then stream in the remaining x chunks round-robin.
            dmaq = (nc.gpsimd, nc.sync, nc.scalar)
            xr = xT.ap().rearrange("(n p) (c t) -> p n c t", p=128, t=TC)
            wq_r = wq_d.ap().rearrange("(n p) m -> p n m", p=128)
            wkv_r = wkv_d.ap().rearrange("(n p) m -> p n m", p=128)
            for dmi in range(NDM):
                r3 = dmi % 3
                dmaq[r3].dma_start(out=x_t[dmi][0][:], in_=xr[:, dmi, 0, :])
                dmaq[(r3 + 1) % 3].dma_start(out=wq_t[dmi][:], in_=wq_r[:, dmi, :])
                dmaq[(r3 + 2) % 3].dma_start(out=wkv_t[dmi][:], in_=wkv_r[:, dmi, :])
            qi = 0
            for c in range(1, NCH):
                for dmi in range(NDM):
                    dmaq[qi % 3].dma_start(out=x_t[dmi][c][:], in_=xr[:, dmi, c, :])
                    qi += 1
            for i in range(NT):
                nc.gpsimd.memset(vA_t[i][:, 256:258], 1.0)

            def zero_diag_regions(c):
                # cols [0, 128j) of diagonal tile 4c+j are read as zero by PV
                # before ever being written; everything else is write-first.
                for j in range(1, 4):
                    nc.gpsimd.memset(pt_sb[:, 4 * c + j, :, 0:128 * j], 0.0)

            zero_diag_regions(0)
            for _ in proj_gen(0):
                pass
            for c in range(NCH):
                filler = proj_gen(c + 1) if c + 1 < NCH else None
                n_steps = 4 * (4 * (c + 1)) + 16
                state = {"step": 0, "emitted": 0}

                def pace():
                    state["step"] += 1
                    if filler is not None:
                        if c < 2:
                            # early chunks: front-load so the serial
                            # projection chains start as soon as possible
                            target = min(N_PIECES, 2 * state["step"])
                        else:
                            target = N_PIECES * state["step"] // n_steps
                        while (state["emitted"] < target
                               and next(filler, "done") != "done"):
                            state["emitted"] += 1

                # h0 scores ascend; PV(h0) frees high kt early (sq desc) so
                # the descending h1 scores' exps unblock during PV(h0); PV(h1)
                # consumes kt descending to start before the last h1 exps.
                if c + 1 < NCH:
                    zero_diag_regions(c + 1)
                emit_scores(c, 0, pace, split_exp=(c == 0))
                pv_out_block(c, 0, pace, sq_desc=True,
                             eager_rs=(c == NCH - 1))
                emit_scores(c, 1, pace, kt_desc=True)
                pv_out_block(c, 1, pace, eager_rs=(c == NCH - 1))
                if filler is not None:
                    for _ in filler:
                        pass
    nc.compile()
    return nc


_NC = None
_last_in_maps = None


def _get_nc():
    global _NC
    if _NC is None:
        _NC = _build()
    return _NC


def kernel(x, Wq, Wk, Wv, lambda_q1, lambda_k1, lambda_q2, lambda_k2,
           softmax_scaler, gn_weight):
    x = np.asarray(x, np.float32)
    Wq = np.asarray(Wq, np.float32)
    Wk = np.asarray(Wk, np.float32)
    Wv = np.asarray(Wv, np.float32)
    lam = float(np.exp(np.sum(np.float64(lambda_q1) * np.float64(lambda_k1)))
                - np.exp(np.sum(np.float64(lambda_q2) * np.float64(lambda_k2)))
                + LAMBDA_INIT)
    softmax_scaler = np.asarray(softmax_scaler, np.float32)
    gn_weight = np.asarray(gn_weight, np.float32)
    logp = np.log(np.arange(1, T + 1, dtype=np.float64)).astype(np.float32)

    nc = _get_nc()
    in_maps = []
    for core in range(8):
        b, r = divmod(core, 4)
        qheads = [2 * r, 2 * r + 1, 8 + 2 * r, 8 + 2 * r + 1]
        wq_c = np.concatenate([Wq[:, hh * 128:(hh + 1) * 128] for hh in qheads], axis=1)
        wkv_c = np.concatenate([
            Wk[:, r * 128:(r + 1) * 128],
            Wk[:, (4 + r) * 128:(5 + r) * 128],
            Wv[:, r * 256:(r + 1) * 256],
        ], axis=1)
        lsc = (logp.reshape(NT, 128).T.reshape(128, NT, 1)
               * softmax_scaler[qheads].reshape(1, 1, 4)).astype(np.float32)
        in_maps.append({
            "xT": np.ascontiguousarray(x[b].T).astype(np.float16),
            "wq": np.ascontiguousarray(wq_c).astype(np.float16),
            "wkv": np.ascontiguousarray(wkv_c).astype(np.float16),
            "lsc": np.ascontiguousarray(lsc),
            "gnw": np.ascontiguousarray(
                np.broadcast_to(gn_weight[2 * r:2 * r + 2].reshape(1, 2, 256), (128, 2, 256))),
            "neglam": np.full((128, 1), -lam, np.float32),
        })
    global _last_in_maps
    _last_in_maps = in_maps
    res = run_bass_kernel_spmd(nc, in_maps, list(range(8)))
    out = np.empty((B, T, 8, 256), np.float32)
    for core in range(8):
        b, r = divmod(core, 4)
        out[b, :, 2 * r:2 * r + 2, :] = res.results[core]["y"]
    return out


# revision 14
# speedup vs baseline: 1.0305x; 1.0305x over previous
"""MixerDiffAttention Trainium2 kernel, v3.

Sharding: 8 cores = batch(2) x head-group(4).  Core (b, r) computes output
heads {2r, 2r+1} of batch b.

v3 over v2 (374us):
 - fp16 x / weights (host-preconverted) and fp16 q/k after rope.  fp16's
   10-bit mantissa keeps the final rel err at ~6e-3 (bf16 would be 2.5e-2).
   No f32r anywhere -> no casting DMAs -> loads can use all 3 DMA queues
   (gpsimd/sync/scalar), LDWEIGHTS drops 188->97ns, transposes 1.5->1.0
   cycles/row, fp32_mode=HIGH power draw is gone, and DMA bytes halve.
 - ALL of x (8MB fp16 = 64 tiles) preloaded at startup across the 3 queues,
   (x-chunk0, wq, wkv) trios rotated first so dmi-d inputs land by ~0.5*d us.
   No mid-kernel x DMA gating at all (v2 stalled ~4us x4 on late x chunks).
 - diagonal score matmuls trimmed to the unmasked column range.
 - ps_p 3 banks so proj of tt+1 isn't gated on tt's rope PSUM reads.
 - eager output normalization for both head blocks of the last chunk.
"""
import numpy as np
import concourse.bass as bass
import concourse.mybir as mybir
from concourse import bacc
from concourse.tile import TileContext
from concourse.bass_utils import run_bass_kernel_spmd

B, T, DM = 2, 2048, 2048
H, KVH, D = 16, 8, 128
TC = 512                  # token chunk (= q chunk)
NT = T // 128             # 16 token tiles
NCH = T // TC             # 4 chunks
NDM = DM // 128           # 16 contraction chunks
EPS = 1e-6
ROPE_BASE = 10000.0
LAMBDA_INIT = 0.8 - 0.6 * np.exp(-0.3 * 12)
F32 = mybir.dt.float32
BF16 = mybir.dt.bfloat16
FP16 = mybir.dt.float16
AF = mybir.ActivationFunctionType
ALU = mybir.AluOpType
AX = mybir.AxisListType
ISQ = float(1.0 / np.sqrt(D))


def _bc_mid(a, n):
    # [128, m] AP -> [128, n(bcast), m]
    return bass.AP(tensor=a.tensor, offset=a.offset, ap=[list(a.ap[0]), [0, n], list(a.ap[1])])


def _bc_last(a, n):
    # [128, m] AP -> [128, m, n(bcast)]
    return bass.AP(tensor=a.tensor, offset=a.offset, ap=[list(a.ap[0]), list(a.ap[1]), [0, n]])


class _Bacc(bacc.Bacc):
    """Bacc that pins every activation to the natural_log_exp_and_others
    table set (it contains exp, ln, square, copy - all this kernel uses),
    so exactly one ACT_TABLE_LOAD is emitted instead of one per ln<->exp
    alternation."""

    def insert_act_table_loads(self):
        import bass_rust as _bass_rust
        from concourse.hw_specs import get_activation_tables

        has_activation = any(
            isinstance(i, mybir.InstActivation)
            for b in self.main_func.blocks
            for i in b.instructions
        )
        if not has_activation:
            return
        keep = "natural_log_exp_and_others"
        tables = [(n, (s if n == keep else set()))
                  for n, s in get_activation_tables(self.m.arch).items()]
        _bass_rust.insert_act_table_loads(self, tables)


def _build():
    nc = _Bacc(None, target_bir_lowering=False)

    xT = nc.dram_tensor("xT", [DM, T], FP16, kind="ExternalInput")
    wq_d = nc.dram_tensor("wq", [128, NDM, 512], FP16, kind="ExternalInput")
    wkv_d = nc.dram_tensor("wkv", [128, NDM, 512], FP16, kind="ExternalInput")
    lsc_d = nc.dram_tensor("lsc", [128, NT, 4], F32, kind="ExternalInput")
    gn_d = nc.dram_tensor("gnw", [128, 2, 256], F32, kind="ExternalInput")
    neglam_d = nc.dram_tensor("neglam", [128, 1], F32, kind="ExternalInput")
    y_d = nc.dram_tensor("y", [T, 2, 256], F32, kind="ExternalOutput")

    pos = np.arange(T, dtype=np.float64)
    inv = ROPE_BASE ** (-np.arange(0, D, 2, dtype=np.float64) / D)  # (64,)
    ang = np.outer(pos, inv)
    cos_h = np.cos(ang).astype(np.float16).reshape(NT, 128, 64).transpose(1, 0, 2).copy()
    sin_h = np.sin(ang).astype(np.float16).reshape(NT, 128, 64).transpose(1, 0, 2).copy()
    pidx = np.arange(128).reshape(128, 1)
    qidx = np.arange(128).reshape(1, 128)
    tri_h = (pidx <= qidx).astype(np.float32)        # keep k<=q in diag block
    ident_h = np.eye(128, dtype=np.float16)

    cos_c = nc.inline_tensor(cos_h, "cos_c")
    sin_c = nc.inline_tensor(sin_h, "sin_c")
    tri_c = nc.inline_tensor(tri_h, "tri_c")
    ident_c = nc.inline_tensor(ident_h, "ident_c")

    with TileContext(nc) as tc:
        with (
            tc.tile_pool(name="wp", bufs=1) as wp,
            tc.tile_pool(name="cp", bufs=1) as cp,
            tc.tile_pool(name="xp", bufs=1) as xp,
            tc.tile_pool(name="kv", bufs=1) as kvp,
            tc.tile_pool(name="qt", bufs=2) as qtp,
            tc.tile_pool(name="wk", bufs=2) as wk,
            tc.tile_pool(name="yo", bufs=1) as yop,
            tc.tile_pool(name="yv", bufs=5) as yvp,
            tc.tile_pool(name="ps_p", bufs=3, space="PSUM") as ps_p,
            tc.tile_pool(name="ps_t", bufs=1, space="PSUM") as ps_t,
            tc.tile_pool(name="ps_x", bufs=4, space="PSUM") as ps_x,
        ):
            # ---- persistent tiles ----
            # weight tiles in groups of 4 dmi (4KB contiguous DRAM rows
            # per partition -> big DMA packets); x as one full-row tile per
            # dmi (xT rows are 4KB contiguous).
            wq_g = [wp.tile([128, 4, 512], FP16, tag=f"wq{g}", name=f"wq{g}")
                    for g in range(NDM // 4)]
            wkv_g = [wp.tile([128, 4, 512], FP16, tag=f"wkv{g}", name=f"wkv{g}")
                     for g in range(NDM // 4)]
            x_t = [xp.tile([128, T], FP16, tag=f"x{d}", name=f"x{d}")
                   for d in range(NDM)]

            cos_sb = cp.tile([128, NT, 64], FP16, tag="cos")
            sin_sb = cp.tile([128, NT, 64], FP16, tag="sin")
            tri_sb = cp.tile([128, 128], BF16, tag="tri")
            ident_sb = cp.tile([128, 128], FP16, tag="ident")
            lsc_sb = cp.tile([128, NT, 4], F32, tag="lsc")
            gn_sb = cp.tile([128, 2, 256], F32, tag="gn")
            neglam_sb = cp.tile([128, 1], F32, tag="neglam")
            eps_sb = cp.tile([128, 1], F32, tag="eps")
            lnisq_sb = cp.tile([128, 1], F32, tag="lnisq")
            nc.sync.dma_start(out=cos_sb, in_=cos_c.ap())
            nc.sync.dma_start(out=sin_sb, in_=sin_c.ap())
            nc.gpsimd.dma_start(out=tri_sb, in_=tri_c.ap())
            nc.gpsimd.dma_start(out=ident_sb, in_=ident_c.ap())
            nc.sync.dma_start(out=lsc_sb, in_=lsc_d.ap())
            nc.sync.dma_start(out=gn_sb, in_=gn_d.ap())
            nc.sync.dma_start(out=neglam_sb, in_=neglam_d.ap())
            nc.vector.memset(eps_sb[:], EPS)
            nc.vector.memset(lnisq_sb[:], float(np.log(ISQ)))

            # persistent per-token-tile K^T (fp16) and V+ones (bf16)
            kT_t = [kvp.tile([128, 2, 128], FP16, tag=f"kT{i}", name=f"kT{i}") for i in range(NT)]
            vA_t = [kvp.tile([128, 258], BF16, tag=f"vA{i}", name=f"vA{i}") for i in range(NT)]
            # probability store [128, kt, s, 512] bf16 (bf16 for range: exp is
            # taken without max-subtraction); zeroed once (below, on gpsimd).
            pt_sb = kvp.tile([128, NT, 2, 512], BF16, tag="pt")
            rsk_sb = kvp.tile([128, NT, 2], F32, tag="rsk")
            ssy_sb = kvp.tile([128, 8], F32, tag="ssy")
            rsy_sb = kvp.tile([128, 8], F32, tag="rsy")

            def proj_gen(c):
                """generator: projection + q/k processing for chunk c, in
                pieces (a yield ~= 4 PE matmuls).  All q matmuls go before all
                kv matmuls: the kv PSUM allocation then doesn't make the
                in-order PE queue wait for the previous tile's rope reads."""
                qT_a = qtp.tile([128, 4, 256], FP16, tag="qta")
                qT_b = qtp.tile([128, 4, 256], FP16, tag="qtb")
                qT_all[c] = (qT_a, qT_b)
                for ti in range(4):
                    tt = c * 4 + ti
                    q_ps = ps_p.tile([128, 512], F32, tag="pp")
                    kv_ps = ps_p.tile([128, 512], F32, tag="pp")
                    for dmi in range(NDM):
                        lhs = x_t[dmi][:, c * TC + ti * 128:c * TC + (ti + 1) * 128]
                        nc.tensor.matmul(q_ps[:], lhs, wq_g[dmi // 4][:, dmi % 4, :],
                                         start=(dmi == 0), stop=(dmi == NDM - 1))
                        if dmi % 2 == 1:
                            yield
                    for dmi in range(NDM):
                        lhs = x_t[dmi][:, c * TC + ti * 128:c * TC + (ti + 1) * 128]
                        nc.tensor.matmul(kv_ps[:], lhs, wkv_g[dmi // 4][:, dmi % 4, :],
                                         start=(dmi == 0), stop=(dmi == NDM - 1))
                        if dmi % 2 == 1:
                            yield

                    # rms stats: scalar square (f32 scratch) -> vector reduce
                    sqq = wk.tile([128, 512], F32, tag="sqq")
                    sqk = wk.tile([128, 256], F32, tag="sqk")
                    nc.scalar.activation(out=sqq[:], in_=q_ps[:], func=AF.Square)
                    nc.scalar.activation(out=sqk[:], in_=kv_ps[:, 0:256], func=AF.Square)
                    ssq = wk.tile([128, 4], F32, tag="ssq")
                    ssk = wk.tile([128, 2], F32, tag="ssk")
                    nc.vector.reduce_sum(ssq[:], sqq[:].rearrange("p (h d) -> p h d", h=4), axis=AX.X)
                    nc.vector.reduce_sum(ssk[:], sqk[:].rearrange("p (h d) -> p h d", h=2), axis=AX.X)
                    lq = wk.tile([128, 4], F32, tag="lq")
                    nc.scalar.activation(out=lq[:], in_=ssq[:], func=AF.Ln,
                                         scale=1.0 / D, bias=eps_sb[:])
                    rsq0 = wk.tile([128, 4], F32, tag="rsq0")
                    nc.scalar.activation(out=rsq0[:], in_=lq[:], func=AF.Exp, scale=-0.5)
                    lk = wk.tile([128, 2], F32, tag="lk")
                    nc.scalar.activation(out=lk[:], in_=ssk[:], func=AF.Ln,
                                         scale=1.0 / D, bias=eps_sb[:])
                    nc.scalar.activation(out=rsk_sb[:, tt, :], in_=lk[:], func=AF.Exp,
                                         scale=-0.5, bias=lnisq_sb[:])
                    # rope reads proj PSUM directly, writes fp16; scale applied
                    # after (rotation commutes with the per-(token,head) scale).
                    # All four PSUM reads go first so the bank frees early.
                    qr = wk.tile([128, 4, 128], FP16, tag="qr")
                    qv = q_ps[:].rearrange("p (h d) -> p h d", h=4)
                    cos4 = _bc_mid(cos_sb[:, tt, :], 4)
                    sin4 = _bc_mid(sin_sb[:, tt, :], 4)
                    t1 = wk.tile([128, 4, 64], F32, tag="t1")
                    t1b = wk.tile([128, 4, 64], F32, tag="t1b")
                    nc.vector.tensor_mul(t1[:], qv[:, :, 64:128], sin4)
                    nc.vector.tensor_mul(t1b[:], qv[:, :, 0:64], sin4)
                    nc.vector.tensor_mul(qr[:, :, 0:64], qv[:, :, 0:64], cos4)
                    nc.vector.tensor_mul(qr[:, :, 64:128], qv[:, :, 64:128], cos4)
                    nc.vector.tensor_add(qr[:, :, 0:64], qr[:, :, 0:64], t1[:])
                    nc.vector.tensor_sub(qr[:, :, 64:128], qr[:, :, 64:128], t1b[:])
                    rsq = wk.tile([128, 4], F32, tag="rsq")
                    nc.vector.tensor_mul(rsq[:], rsq0[:], lsc_sb[:, tt, :])
                    nc.vector.tensor_mul(qr[:], qr[:], _bc_last(rsq[:], 128))
                    yield

                    # v copy (vector); k rope reads PSUM directly, writes fp16
                    nc.vector.tensor_copy(out=vA_t[tt][:, 0:256], in_=kv_ps[:, 256:512])
                    kr = wk.tile([128, 2, 128], FP16, tag="kr")
                    kv_ = kv_ps[:, 0:256].rearrange("p (h d) -> p h d", h=2)
                    cos2 = _bc_mid(cos_sb[:, tt, :], 2)
                    sin2 = _bc_mid(sin_sb[:, tt, :], 2)
                    t2 = t1[:, 0:2, :]
                    t2b = t1b[:, 0:2, :]
                    nc.vector.tensor_mul(t2, kv_[:, :, 64:128], sin2)
                    nc.vector.tensor_mul(t2b, kv_[:, :, 0:64], sin2)
                    nc.vector.tensor_mul(kr[:, :, 0:64], kv_[:, :, 0:64], cos2)
                    nc.vector.tensor_mul(kr[:, :, 64:128], kv_[:, :, 64:128], cos2)
                    nc.vector.tensor_add(kr[:, :, 0:64], kr[:, :, 0:64], t2)
                    nc.vector.tensor_sub(kr[:, :, 64:128], kr[:, :, 64:128], t2b)
                    yield

                    # transposes (PE, fp16, all 6 into one PSUM bank) +
                    # batched PSUM->SBUF copies (vector)
                    tqk = ps_t.tile([128, 768], FP16, tag="tqk")
                    qrf = qr[:].rearrange("p h d -> p (h d)")
                    for h in range(4):
                        nc.tensor.transpose(tqk[:, h * 128:(h + 1) * 128],
                                            qrf[:, h * 128:(h + 1) * 128], ident_sb[:])
                    krf = kr[:].rearrange("p h d -> p (h d)")
                    for h in range(2):
                        nc.tensor.transpose(tqk[:, (4 + h) * 128:(5 + h) * 128],
                                            krf[:, h * 128:(h + 1) * 128], ident_sb[:])
                    qdst = qT_a if ti < 2 else qT_b
                    to = (ti % 2) * 128
                    nc.vector.tensor_copy(
                        out=qdst[:][:, :, to:to + 128],
                        in_=tqk[:, 0:512].rearrange("p (h t) -> p h t", h=4))
                    nc.vector.tensor_copy(out=kT_t[tt][:], in_=tqk[:, 512:768].rearrange("p (h t) -> p h t", h=2))
                    yield

            qT_all = {}
            dmaq3 = (nc.sync, nc.gpsimd, nc.scalar)

            def pv_out_block(c, h, pace=None, sq_desc=False, kt_desc=False,
                             eager_rs=False):
                """PV accumulation + output combine for chunk c, head h."""
                yv_tiles = {}
                sqs = (3, 2, 1, 0) if sq_desc else (0, 1, 2, 3)
                for sq in sqs:
                    qt_g = 4 * c + sq
                    o0 = ps_x.tile([128, 258], F32, tag="st")
                    o1 = ps_x.tile([128, 258], F32, tag="st")
                    for s, o in ((0, o0), (1, o1)):
                        nkt = qt_g + 1
                        kts = range(nkt - 1, -1, -1) if kt_desc else range(nkt)
                        for i, kt in enumerate(kts):
                            nc.tensor.matmul(
                                o[:], pt_sb[:, kt, s, sq * 128:(sq + 1) * 128],
                                vA_t[kt][:], start=(i == 0), stop=(i == nkt - 1))
                        if pace is not None:
                            pace()
                    r0 = yop.tile([128, 1], F32, tag="r0")
                    r1 = yop.tile([128, 1], F32, tag="r1")
                    nc.vector.reciprocal(r0[:], o0[:, 256:257])
                    nc.vector.reciprocal(r1[:], o1[:, 256:257])
                    nc.vector.tensor_mul(r1[:], r1[:], neglam_sb[:])
                    tsb = yop.tile([128, 256], F32, tag="t")
                    nc.vector.tensor_scalar_mul(tsb[:], o1[:, 0:256], r1[:])
                    yv = yvp.tile([128, 256], F32, tag="yv")
                    yv_tiles[sq] = yv
                    nc.vector.scalar_tensor_tensor(
                        out=yv[:], in0=o0[:, 0:256], scalar=r0[:], in1=tsb[:],
                        op0=ALU.mult, op1=ALU.add)
                    nc.scalar.activation(out=tsb[:], in_=yv[:], func=AF.Square)
                    nc.vector.reduce_sum(ssy_sb[:, 4 * h + sq:4 * h + sq + 1],
                                         tsb[:], axis=AX.X)
                    if eager_rs:
                        i0 = 4 * h + sq
                        ly1 = yop.tile([128, 1], F32, tag="ly1")
                        nc.scalar.activation(out=ly1[:], in_=ssy_sb[:, i0:i0 + 1],
                                             func=AF.Ln, scale=1.0 / 256, bias=eps_sb[:])
                        nc.scalar.activation(out=rsy_sb[:, i0:i0 + 1], in_=ly1[:],
                                             func=AF.Exp, scale=-0.5)
                        yo = yop.tile([128, 256], F32, tag="yo")
                        nc.vector.scalar_tensor_tensor(
                            out=yo[:], in0=yv[:], scalar=rsy_sb[:, i0:i0 + 1],
                            in1=gn_sb[:, h, :], op0=ALU.mult, op1=ALU.mult)
                        dmaq3[(2 * c + h + sq) % 3].dma_start(
                            out=y_d.ap()[qt_g * 128:(qt_g + 1) * 128, h, :],
                            in_=yo[:])
                if eager_rs:
                    return
                ly = yop.tile([128, 4], F32, tag="ly")
                nc.scalar.activation(out=ly[:], in_=ssy_sb[:, 4 * h:4 * h + 4],
                                     func=AF.Ln, scale=1.0 / 256, bias=eps_sb[:])
                nc.scalar.activation(out=rsy_sb[:, 4 * h:4 * h + 4], in_=ly[:],
                                     func=AF.Exp, scale=-0.5)
                for sq in range(4):
                    qt_g = 4 * c + sq
                    yo = yop.tile([128, 256], F32, tag="yo")
                    nc.vector.scalar_tensor_tensor(
                        out=yo[:], in0=yv_tiles[sq][:],
                        scalar=rsy_sb[:, 4 * h + sq:4 * h + sq + 1],
                        in1=gn_sb[:, h, :], op0=ALU.mult, op1=ALU.mult)
                    dmaq3[(2 * c + h + sq) % 3].dma_start(
                        out=y_d.ap()[qt_g * 128:(qt_g + 1) * 128, h, :],
                        in_=yo[:])

            def emit_scores(c, h, pace, kt_desc=False, split_exp=False):
                """score matmul + exp for (chunk c, head h), interleaving
                filler pieces (projection of c+1) between steps.  Each score
                matmul is split at column 256 (the qT_a/qT_b tile boundary) so
                the first half only depends on the chunk's first two token
                tiles."""
                qT_a, qT_b = qT_all[c]
                kts = (range(4 * (c + 1) - 1, -1, -1) if kt_desc
                       else range(4 * (c + 1)))
                steps = [(kt, s) for kt in kts for s in range(2)]
                for (kt, s) in steps:
                    j = kt - 4 * c
                    L = max(j, 0) * 128
                    st = ps_x.tile([128, 512], F32, tag="st")
                    if L < 256:
                        nc.tensor.matmul(st[:, L:256], kT_t[kt][:, s, :],
                                         qT_a[:, 2 * s + h, L:256],
                                         start=True, stop=True)
                    nc.tensor.matmul(st[:, max(L, 256):512], kT_t[kt][:, s, :],
                                     qT_b[:, 2 * s + h, max(L, 256) - 256:256],
                                     start=True, stop=True)
                    if split_exp and L < 256:
                        nc.scalar.activation(
                            out=pt_sb[:, kt, s, L:256], in_=st[:, L:256],
                            func=AF.Exp, scale=rsk_sb[:, kt, s:s + 1])
                        nc.scalar.activation(
                            out=pt_sb[:, kt, s, 256:512], in_=st[:, 256:512],
                            func=AF.Exp, scale=rsk_sb[:, kt, s:s + 1])
                    else:
                        nc.scalar.activation(
                            out=pt_sb[:, kt, s, L:512], in_=st[:, L:512],
                            func=AF.Exp, scale=rsk_sb[:, kt, s:s + 1])
                    if j >= 0:
                        nc.vector.tensor_mul(
                            pt_sb[:, kt, s, L:L + 128],
                            pt_sb[:, kt, s, L:L + 128], tri_sb[:])
                    pace()

            # ---------------- main schedule ----------------
            N_PIECES = 4 * (8 + 8 + 3)
            # startup DMAs: 16 x-row DMAs (512KB, 4KB/partition lines) and
            # 8 weight-group DMAs (1MB, 4KB lines), round-robin over the 3
            # DMA queues in need order: weight group g interleaved with the x
            # rows its dmi range consumes.
            dmaq = (nc.gpsimd, nc.sync, nc.scalar)
            xr = xT.ap().rearrange("(n p) t -> p n t", p=128)
            items = []
            for g in range(NDM // 4):
                items.append((wq_g[g][:], wq_d.ap()[:, 4 * g:4 * g + 4, :]))
                items.append((wkv_g[g][:], wkv_d.ap()[:, 4 * g:4 * g + 4, :]))
                for dmi in range(4 * g, 4 * g + 4):
                    items.append((x_t[dmi][:], xr[:, dmi, :]))
            for i, (o, inp) in enumerate(items):
                dmaq[i % 3].dma_start(out=o, in_=inp)
            for i in range(NT):
                nc.gpsimd.memset(vA_t[i][:, 256:258], 1.0)

            def zero_diag_regions(c):
                # cols [0, 128j) of diagonal tile 4c+j are read as zero by PV
                # before ever being written; everything else is write-first.
                for j in range(1, 4):
                    nc.gpsimd.memset(pt_sb[:, 4 * c + j, :, 0:128 * j], 0.0)

            zero_diag_regions(0)
            for _ in proj_gen(0):
                pass
            for c in range(NCH):
                filler = proj_gen(c + 1) if c + 1 < NCH else None
                n_steps = 4 * (4 * (c + 1)) + 16
                state = {"step": 0, "emitted": 0}

                def pace():
                    state["step"] += 1
                    if filler is not None:
                        if c < 2:
                            # early chunks: front-load so the serial
                            # projection chains start as soon as possible
                            target = min(N_PIECES, 2 * state["step"])
                        else:
                            target = N_PIECES * state["step"] // n_steps
                        while (state["emitted"] < target
                               and next(filler, "done") != "done"):
                            state["emitted"] += 1

                # h0 scores ascend; PV(h0) frees high kt early (sq desc) so
                # the descending h1 scores' exps unblock during PV(h0); PV(h1)
                # consumes kt descending to start before the last h1 exps.
                if c + 1 < NCH:
                    zero_diag_regions(c + 1)
                emit_scores(c, 0, pace, split_exp=(c == 0))
                pv_out_block(c, 0, pace, sq_desc=True,
                             eager_rs=(c == NCH - 1))
                emit_scores(c, 1, pace, kt_desc=True)
                pv_out_block(c, 1, pace, eager_rs=(c == NCH - 1))
                if filler is not None:
                    for _ in filler:
                        pass
    nc.compile()
    return nc


_NC = None
_last_in_maps = None


def _get_nc():
    global _NC
    if _NC is None:
        _NC = _build()
    return _NC


def kernel(x, Wq, Wk, Wv, lambda_q1, lambda_k1, lambda_q2, lambda_k2,
           softmax_scaler, gn_weight):
    x = np.asarray(x, np.float32)
    Wq = np.asarray(Wq, np.float32)
    Wk = np.asarray(Wk, np.float32)
    Wv = np.asarray(Wv, np.float32)
    lam = float(np.exp(np.sum(np.float64(lambda_q1) * np.float64(lambda_k1)))
                - np.exp(np.sum(np.float64(lambda_q2) * np.float64(lambda_k2)))
                + LAMBDA_INIT)
    softmax_scaler = np.asarray(softmax_scaler, np.float32)
    gn_weight = np.asarray(gn_weight, np.float32)
    logp = np.log(np.arange(1, T + 1, dtype=np.float64)).astype(np.float32)

    nc = _get_nc()
    in_maps = []
    for core in range(8):
        b, r = divmod(core, 4)
        qheads = [2 * r, 2 * r + 1, 8 + 2 * r, 8 + 2 * r + 1]
        wq_c = np.concatenate([Wq[:, hh * 128:(hh + 1) * 128] for hh in qheads], axis=1)
        wkv_c = np.concatenate([
            Wk[:, r * 128:(r + 1) * 128],
            Wk[:, (4 + r) * 128:(5 + r) * 128],
            Wv[:, r * 256:(r + 1) * 256],
        ], axis=1)
        lsc = (logp.reshape(NT, 128).T.reshape(128, NT, 1)
               * softmax_scaler[qheads].reshape(1, 1, 4)).astype(np.float32)
        wq_p = wq_c.reshape(NDM, 128, 512).transpose(1, 0, 2)
        wkv_p = wkv_c.reshape(NDM, 128, 512).transpose(1, 0, 2)
        in_maps.append({
            "xT": np.ascontiguousarray(x[b].T).astype(np.float16),
            "wq": np.ascontiguousarray(wq_p).astype(np.float16),
            "wkv": np.ascontiguousarray(wkv_p).astype(np.float16),
            "lsc": np.ascontiguousarray(lsc),
            "gnw": np.ascontiguousarray(
                np.broadcast_to(gn_weight[2 * r:2 * r + 2].reshape(1, 2, 256), (128, 2, 256))),
            "neglam": np.full((128, 1), -lam, np.float32),
        })
    global _last_in_maps
    _last_in_maps = in_maps
    res = run_bass_kernel_spmd(nc, in_maps, list(range(8)))
    out = np.empty((B, T, 8, 256), np.float32)
    for core in range(8):
        b, r = divmod(core, 4)
        out[b, :, 2 * r:2 * r + 2, :] = res.results[core]["y"]
    return out


# revision 16
# speedup vs baseline: 1.0595x; 1.0281x over previous
"""MixerDiffAttention Trainium2 kernel, v3.

Sharding: 8 cores = batch(2) x head-group(4).  Core (b, r) computes output
heads {2r, 2r+1} of batch b.

v3 over v2 (374us):
 - fp16 x / weights (host-preconverted) and fp16 q/k after rope.  fp16's
   10-bit mantissa keeps the final rel err at ~6e-3 (bf16 would be 2.5e-2).
   No f32r anywhere -> no casting DMAs -> loads can use all 3 DMA queues
   (gpsimd/sync/scalar), LDWEIGHTS drops 188->97ns, transposes 1.5->1.0
   cycles/row, fp32_mode=HIGH power draw is gone, and DMA bytes halve.
 - ALL of x (8MB fp16 = 64 tiles) preloaded at startup across the 3 queues,
   (x-chunk0, wq, wkv) trios rotated first so dmi-d inputs land by ~0.5*d us.
   No mid-kernel x DMA gating at all (v2 stalled ~4us x4 on late x chunks).
 - diagonal score matmuls trimmed to the unmasked column range.
 - ps_p 3 banks so proj of tt+1 isn't gated on tt's rope PSUM reads.
 - eager output normalization for both head blocks of the last chunk.
"""
import numpy as np
import concourse.bass as bass
import concourse.mybir as mybir
from concourse import bacc
from concourse.tile import TileContext
from concourse.bass_utils import run_bass_kernel_spmd

B, T, DM = 2, 2048, 2048
H, KVH, D = 16, 8, 128
TC = 512                  # token chunk (= q chunk)
NT = T // 128             # 16 token tiles
NCH = T // TC             # 4 chunks
NDM = DM // 128           # 16 contraction chunks
EPS = 1e-6
ROPE_BASE = 10000.0
LAMBDA_INIT = 0.8 - 0.6 * np.exp(-0.3 * 12)
F32 = mybir.dt.float32
BF16 = mybir.dt.bfloat16
FP16 = mybir.dt.float16
AF = mybir.ActivationFunctionType
ALU = mybir.AluOpType
AX = mybir.AxisListType
ISQ = float(1.0 / np.sqrt(D))


def _bc_mid(a, n):
    # [128, m] AP -> [128, n(bcast), m]
    return bass.AP(tensor=a.tensor, offset=a.offset, ap=[list(a.ap[0]), [0, n], list(a.ap[1])])


def _bc_last(a, n):
    # [128, m] AP -> [128, m, n(bcast)]
    return bass.AP(tensor=a.tensor, offset=a.offset, ap=[list(a.ap[0]), list(a.ap[1]), [0, n]])


class _Bacc(bacc.Bacc):
    """Bacc that pins every activation to the natural_log_exp_and_others
    table set (it contains exp, ln, square, copy - all this kernel uses),
    so exactly one ACT_TABLE_LOAD is emitted instead of one per ln<->exp
    alternation."""

    def insert_act_table_loads(self):
        import bass_rust as _bass_rust
        from concourse.hw_specs import get_activation_tables

        has_activation = any(
            isinstance(i, mybir.InstActivation)
            for b in self.main_func.blocks
            for i in b.instructions
        )
        if not has_activation:
            return
        keep = "natural_log_exp_and_others"
        tables = [(n, (s if n == keep else set()))
                  for n, s in get_activation_tables(self.m.arch).items()]
        _bass_rust.insert_act_table_loads(self, tables)


def _build():
    nc = _Bacc(None, target_bir_lowering=False)

    xT = nc.dram_tensor("xT", [NCH, NDM // 4, 128, 4, TC], FP16, kind="ExternalInput")
    wq_d = nc.dram_tensor("wq", [128, NDM, 512], FP16, kind="ExternalInput")
    wkv_d = nc.dram_tensor("wkv", [128, NDM, 512], FP16, kind="ExternalInput")
    lsc_d = nc.dram_tensor("lsc", [128, NT, 4], F32, kind="ExternalInput")
    gn_d = nc.dram_tensor("gnw", [128, 2, 256], F32, kind="ExternalInput")
    neglam_d = nc.dram_tensor("neglam", [128, 1], F32, kind="ExternalInput")
    y_d = nc.dram_tensor("y", [T, 2, 256], FP16, kind="ExternalOutput")

    pos = np.arange(T, dtype=np.float64)
    inv = ROPE_BASE ** (-np.arange(0, D, 2, dtype=np.float64) / D)  # (64,)
    ang = np.outer(pos, inv)
    cos_h = np.cos(ang).astype(np.float16).reshape(NT, 128, 64).transpose(1, 0, 2).copy()
    sin_h = np.sin(ang).astype(np.float16).reshape(NT, 128, 64).transpose(1, 0, 2).copy()
    pidx = np.arange(128).reshape(128, 1)
    qidx = np.arange(128).reshape(1, 128)
    tri_h = (pidx <= qidx).astype(np.float32)        # keep k<=q in diag block
    ident_h = np.eye(128, dtype=np.float16)

    cos_c = nc.inline_tensor(cos_h, "cos_c")
    sin_c = nc.inline_tensor(sin_h, "sin_c")
    tri_c = nc.inline_tensor(tri_h, "tri_c")
    ident_c = nc.inline_tensor(ident_h, "ident_c")

    with TileContext(nc) as tc:
        with (
            tc.tile_pool(name="wp", bufs=1) as wp,
            tc.tile_pool(name="cp", bufs=1) as cp,
            tc.tile_pool(name="xp", bufs=1) as xp,
            tc.tile_pool(name="kv", bufs=1) as kvp,
            tc.tile_pool(name="qt", bufs=2) as qtp,
            tc.tile_pool(name="wk", bufs=2) as wk,
            tc.tile_pool(name="yo", bufs=1) as yop,
            tc.tile_pool(name="yv", bufs=5) as yvp,
            tc.tile_pool(name="ps_p", bufs=3, space="PSUM") as ps_p,
            tc.tile_pool(name="ps_t", bufs=1, space="PSUM") as ps_t,
            tc.tile_pool(name="ps_x", bufs=4, space="PSUM") as ps_x,
        ):
            # ---- persistent tiles ----
            # weight tiles in groups of 4 dmi (4KB contiguous DRAM rows
            # per partition -> big DMA packets); x as one full-row tile per
            # dmi (xT rows are 4KB contiguous).
            wq_g = [wp.tile([128, 4, 512], FP16, tag=f"wq{g}", name=f"wq{g}")
                    for g in range(NDM // 4)]
            wkv_g = [wp.tile([128, 4, 512], FP16, tag=f"wkv{g}", name=f"wkv{g}")
                     for g in range(NDM // 4)]
            x_t = [[xp.tile([128, 4, TC], FP16, tag=f"x{c}_{g}", name=f"x{c}_{g}")
                    for g in range(NDM // 4)] for c in range(NCH)]

            cos_sb = cp.tile([128, NT, 64], FP16, tag="cos")
            sin_sb = cp.tile([128, NT, 64], FP16, tag="sin")
            tri_sb = cp.tile([128, 128], BF16, tag="tri")
            ident_sb = cp.tile([128, 128], FP16, tag="ident")
            lsc_sb = cp.tile([128, NT, 4], F32, tag="lsc")
            gn_sb = cp.tile([128, 2, 256], F32, tag="gn")
            neglam_sb = cp.tile([128, 1], F32, tag="neglam")
            eps_sb = cp.tile([128, 1], F32, tag="eps")
            lnisq_sb = cp.tile([128, 1], F32, tag="lnisq")
            nc.sync.dma_start(out=cos_sb, in_=cos_c.ap())
            nc.sync.dma_start(out=sin_sb, in_=sin_c.ap())
            nc.gpsimd.dma_start(out=tri_sb, in_=tri_c.ap())
            nc.gpsimd.dma_start(out=ident_sb, in_=ident_c.ap())
            nc.sync.dma_start(out=lsc_sb, in_=lsc_d.ap())
            nc.sync.dma_start(out=gn_sb, in_=gn_d.ap())
            nc.sync.dma_start(out=neglam_sb, in_=neglam_d.ap())
            nc.vector.memset(eps_sb[:], EPS)
            nc.vector.memset(lnisq_sb[:], float(np.log(ISQ)))

            # persistent per-token-tile K^T (fp16) and V+ones (bf16)
            kT_t = [kvp.tile([128, 2, 128], FP16, tag=f"kT{i}", name=f"kT{i}") for i in range(NT)]
            vA_t = [kvp.tile([128, 258], BF16, tag=f"vA{i}", name=f"vA{i}") for i in range(NT)]
            # probability store [128, kt, s, 512] bf16 (bf16 for range: exp is
            # taken without max-subtraction); zeroed once (below, on gpsimd).
            pt_sb = kvp.tile([128, NT, 2, 512], BF16, tag="pt")
            rsk_sb = kvp.tile([128, NT, 2], F32, tag="rsk")
            ssy_sb = kvp.tile([128, 8], F32, tag="ssy")
            rsy_sb = kvp.tile([128, 8], F32, tag="rsy")

            def proj_gen(c):
                """generator: projection + q/k processing for chunk c, in
                pieces (a yield ~= 4 PE matmuls).  All q matmuls go before all
                kv matmuls: the kv PSUM allocation then doesn't make the
                in-order PE queue wait for the previous tile's rope reads."""
                qT_a = qtp.tile([128, 4, 256], FP16, tag="qta")
                qT_b = qtp.tile([128, 4, 256], FP16, tag="qtb")
                qT_all[c] = (qT_a, qT_b)
                for ti in range(4):
                    tt = c * 4 + ti
                    q_ps = ps_p.tile([128, 512], F32, tag="pp")
                    kv_ps = ps_p.tile([128, 512], F32, tag="pp")
                    for dmi in range(NDM):
                        lhs = x_t[c][dmi // 4][:, dmi % 4, ti * 128:(ti + 1) * 128]
                        nc.tensor.matmul(q_ps[:], lhs, wq_g[dmi // 4][:, dmi % 4, :],
                                         start=(dmi == 0), stop=(dmi == NDM - 1))
                        if dmi % 2 == 1:
                            yield
                    for dmi in range(NDM):
                        lhs = x_t[c][dmi // 4][:, dmi % 4, ti * 128:(ti + 1) * 128]
                        nc.tensor.matmul(kv_ps[:], lhs, wkv_g[dmi // 4][:, dmi % 4, :],
                                         start=(dmi == 0), stop=(dmi == NDM - 1))
                        if dmi % 2 == 1:
                            yield

                    # rms stats: scalar square (f32 scratch) -> vector reduce
                    sqq = wk.tile([128, 512], F32, tag="sqq")
                    sqk = wk.tile([128, 256], F32, tag="sqk")
                    nc.scalar.activation(out=sqq[:], in_=q_ps[:], func=AF.Square)
                    nc.scalar.activation(out=sqk[:], in_=kv_ps[:, 0:256], func=AF.Square)
                    ssq = wk.tile([128, 4], F32, tag="ssq")
                    ssk = wk.tile([128, 2], F32, tag="ssk")
                    nc.vector.reduce_sum(ssq[:], sqq[:].rearrange("p (h d) -> p h d", h=4), axis=AX.X)
                    nc.vector.reduce_sum(ssk[:], sqk[:].rearrange("p (h d) -> p h d", h=2), axis=AX.X)
                    lq = wk.tile([128, 4], F32, tag="lq")
                    nc.scalar.activation(out=lq[:], in_=ssq[:], func=AF.Ln,
                                         scale=1.0 / D, bias=eps_sb[:])
                    rsq0 = wk.tile([128, 4], F32, tag="rsq0")
                    nc.scalar.activation(out=rsq0[:], in_=lq[:], func=AF.Exp, scale=-0.5)
                    lk = wk.tile([128, 2], F32, tag="lk")
                    nc.scalar.activation(out=lk[:], in_=ssk[:], func=AF.Ln,
                                         scale=1.0 / D, bias=eps_sb[:])
                    nc.scalar.activation(out=rsk_sb[:, tt, :], in_=lk[:], func=AF.Exp,
                                         scale=-0.5, bias=lnisq_sb[:])
                    # rope reads proj PSUM directly, writes fp16; scale applied
                    # after (rotation commutes with the per-(token,head) scale).
                    # All four PSUM reads go first so the bank frees early.
                    qr = wk.tile([128, 4, 128], FP16, tag="qr")
                    qv = q_ps[:].rearrange("p (h d) -> p h d", h=4)
                    cos4 = _bc_mid(cos_sb[:, tt, :], 4)
                    sin4 = _bc_mid(sin_sb[:, tt, :], 4)
                    t1 = wk.tile([128, 4, 64], F32, tag="t1")
                    t1b = wk.tile([128, 4, 64], F32, tag="t1b")
                    nc.vector.tensor_mul(t1[:], qv[:, :, 64:128], sin4)
                    nc.vector.tensor_mul(t1b[:], qv[:, :, 0:64], sin4)
                    nc.vector.tensor_mul(qr[:, :, 0:64], qv[:, :, 0:64], cos4)
                    nc.vector.tensor_mul(qr[:, :, 64:128], qv[:, :, 64:128], cos4)
                    nc.vector.tensor_add(qr[:, :, 0:64], qr[:, :, 0:64], t1[:])
                    nc.vector.tensor_sub(qr[:, :, 64:128], qr[:, :, 64:128], t1b[:])
                    rsq = wk.tile([128, 4], F32, tag="rsq")
                    nc.vector.tensor_mul(rsq[:], rsq0[:], lsc_sb[:, tt, :])
                    nc.vector.tensor_mul(qr[:], qr[:], _bc_last(rsq[:], 128))
                    yield

                    # v copy (vector); k rope reads PSUM directly, writes fp16
                    nc.vector.tensor_copy(out=vA_t[tt][:, 0:256], in_=kv_ps[:, 256:512])
                    kr = wk.tile([128, 2, 128], FP16, tag="kr")
                    kv_ = kv_ps[:, 0:256].rearrange("p (h d) -> p h d", h=2)
                    cos2 = _bc_mid(cos_sb[:, tt, :], 2)
                    sin2 = _bc_mid(sin_sb[:, tt, :], 2)
                    t2 = t1[:, 0:2, :]
                    t2b = t1b[:, 0:2, :]
                    nc.vector.tensor_mul(t2, kv_[:, :, 64:128], sin2)
                    nc.vector.tensor_mul(t2b, kv_[:, :, 0:64], sin2)
                    nc.vector.tensor_mul(kr[:, :, 0:64], kv_[:, :, 0:64], cos2)
                    nc.vector.tensor_mul(kr[:, :, 64:128], kv_[:, :, 64:128], cos2)
                    nc.vector.tensor_add(kr[:, :, 0:64], kr[:, :, 0:64], t2)
                    nc.vector.tensor_sub(kr[:, :, 64:128], kr[:, :, 64:128], t2b)
                    yield

                    # transposes (PE, fp16, all 6 into one PSUM bank) +
                    # batched PSUM->SBUF copies (vector)
                    tqk = ps_t.tile([128, 768], FP16, tag="tqk")
                    qrf = qr[:].rearrange("p h d -> p (h d)")
                    for h in range(4):
                        nc.tensor.transpose(tqk[:, h * 128:(h + 1) * 128],
                                            qrf[:, h * 128:(h + 1) * 128], ident_sb[:])
                    krf = kr[:].rearrange("p h d -> p (h d)")
                    for h in range(2):
                        nc.tensor.transpose(tqk[:, (4 + h) * 128:(5 + h) * 128],
                                            krf[:, h * 128:(h + 1) * 128], ident_sb[:])
                    qdst = qT_a if ti < 2 else qT_b
                    to = (ti % 2) * 128
                    nc.vector.tensor_copy(
                        out=qdst[:][:, :, to:to + 128],
                        in_=tqk[:, 0:512].rearrange("p (h t) -> p h t", h=4))
                    nc.vector.tensor_copy(out=kT_t[tt][:], in_=tqk[:, 512:768].rearrange("p (h t) -> p h t", h=2))
                    yield

            qT_all = {}
            dmaq3 = (nc.sync, nc.gpsimd, nc.scalar)

            def pv_out_block(c, h, pace=None, sq_desc=False, kt_desc=False,
                             eager_rs=False):
                """PV accumulation + output combine for chunk c, head h."""
                yv_tiles = {}
                sqs = (3, 2, 1, 0) if sq_desc else (0, 1, 2, 3)
                for sq in sqs:
                    qt_g = 4 * c + sq
                    o0 = ps_x.tile([128, 258], F32, tag="st")
                    o1 = ps_x.tile([128, 258], F32, tag="st")
                    for s, o in ((0, o0), (1, o1)):
                        nkt = qt_g + 1
                        kts = range(nkt - 1, -1, -1) if kt_desc else range(nkt)
                        for i, kt in enumerate(kts):
                            nc.tensor.matmul(
                                o[:], pt_sb[:, kt, s, sq * 128:(sq + 1) * 128],
                                vA_t[kt][:], start=(i == 0), stop=(i == nkt - 1))
                        if pace is not None:
                            pace()
                    r0 = yop.tile([128, 1], F32, tag="r0")
                    r1 = yop.tile([128, 1], F32, tag="r1")
                    nc.vector.reciprocal(r0[:], o0[:, 256:257])
                    nc.vector.reciprocal(r1[:], o1[:, 256:257])
                    nc.vector.tensor_mul(r1[:], r1[:], neglam_sb[:])
                    tsb = yop.tile([128, 256], F32, tag="t")
                    nc.vector.tensor_scalar_mul(tsb[:], o1[:, 0:256], r1[:])
                    yv = yvp.tile([128, 256], F32, tag="yv")
                    yv_tiles[sq] = yv
                    nc.vector.scalar_tensor_tensor(
                        out=yv[:], in0=o0[:, 0:256], scalar=r0[:], in1=tsb[:],
                        op0=ALU.mult, op1=ALU.add)
                    nc.scalar.activation(out=tsb[:], in_=yv[:], func=AF.Square)
                    nc.vector.reduce_sum(ssy_sb[:, 4 * h + sq:4 * h + sq + 1],
                                         tsb[:], axis=AX.X)
                    if eager_rs:
                        i0 = 4 * h + sq
                        ly1 = yop.tile([128, 1], F32, tag="ly1")
                        nc.scalar.activation(out=ly1[:], in_=ssy_sb[:, i0:i0 + 1],
                                             func=AF.Ln, scale=1.0 / 256, bias=eps_sb[:])
                        nc.scalar.activation(out=rsy_sb[:, i0:i0 + 1], in_=ly1[:],
                                             func=AF.Exp, scale=-0.5)
                        yo = yop.tile([128, 256], FP16, tag="yo")
                        nc.vector.scalar_tensor_tensor(
                            out=yo[:], in0=yv[:], scalar=rsy_sb[:, i0:i0 + 1],
                            in1=gn_sb[:, h, :], op0=ALU.mult, op1=ALU.mult)
                        dmaq3[(2 * c + h + sq) % 3].dma_start(
                            out=y_d.ap()[qt_g * 128:(qt_g + 1) * 128, h, :],
                            in_=yo[:])
                if eager_rs:
                    return
                ly = yop.tile([128, 4], F32, tag="ly")
                nc.scalar.activation(out=ly[:], in_=ssy_sb[:, 4 * h:4 * h + 4],
                                     func=AF.Ln, scale=1.0 / 256, bias=eps_sb[:])
                nc.scalar.activation(out=rsy_sb[:, 4 * h:4 * h + 4], in_=ly[:],
                                     func=AF.Exp, scale=-0.5)
                for sq in range(4):
                    qt_g = 4 * c + sq
                    yo = yop.tile([128, 256], FP16, tag="yo")
                    nc.vector.scalar_tensor_tensor(
                        out=yo[:], in0=yv_tiles[sq][:],
                        scalar=rsy_sb[:, 4 * h + sq:4 * h + sq + 1],
                        in1=gn_sb[:, h, :], op0=ALU.mult, op1=ALU.mult)
                    dmaq3[(2 * c + h + sq) % 3].dma_start(
                        out=y_d.ap()[qt_g * 128:(qt_g + 1) * 128, h, :],
                        in_=yo[:])

            def emit_scores(c, h, pace, kt_desc=False, split_exp=False):
                """score matmul + exp for (chunk c, head h), interleaving
                filler pieces (projection of c+1) between steps.  Each score
                matmul is split at column 256 (the qT_a/qT_b tile boundary) so
                the first half only depends on the chunk's first two token
                tiles."""
                qT_a, qT_b = qT_all[c]
                kts = (range(4 * (c + 1) - 1, -1, -1) if kt_desc
                       else range(4 * (c + 1)))
                steps = [(kt, s) for kt in kts for s in range(2)]
                for (kt, s) in steps:
                    j = kt - 4 * c
                    L = max(j, 0) * 128
                    st = ps_x.tile([128, 512], F32, tag="st")
                    if L < 256:
                        nc.tensor.matmul(st[:, L:256], kT_t[kt][:, s, :],
                                         qT_a[:, 2 * s + h, L:256],
                                         start=True, stop=True)
                    nc.tensor.matmul(st[:, max(L, 256):512], kT_t[kt][:, s, :],
                                     qT_b[:, 2 * s + h, max(L, 256) - 256:256],
                                     start=True, stop=True)
                    if split_exp and L < 256:
                        nc.scalar.activation(
                            out=pt_sb[:, kt, s, L:256], in_=st[:, L:256],
                            func=AF.Exp, scale=rsk_sb[:, kt, s:s + 1])
                        nc.scalar.activation(
                            out=pt_sb[:, kt, s, 256:512], in_=st[:, 256:512],
                            func=AF.Exp, scale=rsk_sb[:, kt, s:s + 1])
                    else:
                        nc.scalar.activation(
                            out=pt_sb[:, kt, s, L:512], in_=st[:, L:512],
                            func=AF.Exp, scale=rsk_sb[:, kt, s:s + 1])
                    if j >= 0:
                        nc.vector.tensor_mul(
                            pt_sb[:, kt, s, L:L + 128],
                            pt_sb[:, kt, s, L:L + 128], tri_sb[:])
                    pace()

            # ---------------- main schedule ----------------
            N_PIECES = 4 * (8 + 8 + 3)
            # startup DMAs, all with 4KB/partition contiguous lines.
            # Priority: (wq_g, wkv_g, x0_g) per dmi-group g (chunk-0's proj
            # consumes exactly these in g order), then chunks 1-3 of x.
            dmaq = (nc.gpsimd, nc.sync, nc.scalar)
            items = []
            for g in range(NDM // 4):
                items.append((wq_g[g][:], wq_d.ap()[:, 4 * g:4 * g + 4, :]))
                items.append((wkv_g[g][:], wkv_d.ap()[:, 4 * g:4 * g + 4, :]))
                items.append((x_t[0][g][:], xT.ap()[0, g]))
            for c in range(1, NCH):
                for g in range(NDM // 4):
                    items.append((x_t[c][g][:], xT.ap()[c, g]))
            for i, (o, inp) in enumerate(items):
                dmaq[i % 3].dma_start(out=o, in_=inp)
            for i in range(NT):
                nc.gpsimd.memset(vA_t[i][:, 256:258], 1.0)

            def zero_diag_regions(c):
                # cols [0, 128j) of diagonal tile 4c+j are read as zero by PV
                # before ever being written; everything else is write-first.
                for j in range(1, 4):
                    nc.gpsimd.memset(pt_sb[:, 4 * c + j, :, 0:128 * j], 0.0)

            zero_diag_regions(0)
            for _ in proj_gen(0):
                pass
            for c in range(NCH):
                filler = proj_gen(c + 1) if c + 1 < NCH else None
                n_steps = 4 * (4 * (c + 1)) + 16
                state = {"step": 0, "emitted": 0}

                def pace():
                    state["step"] += 1
                    if filler is not None:
                        if c < 2:
                            # early chunks: front-load so the serial
                            # projection chains start as soon as possible
                            target = min(N_PIECES, 2 * state["step"])
                        else:
                            target = N_PIECES * state["step"] // n_steps
                        while (state["emitted"] < target
                               and next(filler, "done") != "done"):
                            state["emitted"] += 1

                # h0 scores ascend; PV(h0) frees high kt early (sq desc) so
                # the descending h1 scores' exps unblock during PV(h0); PV(h1)
                # consumes kt descending to start before the last h1 exps.
                if c + 1 < NCH:
                    zero_diag_regions(c + 1)
                emit_scores(c, 0, pace, split_exp=(c == 0))
                pv_out_block(c, 0, pace, sq_desc=True,
                             eager_rs=(c == NCH - 1))
                emit_scores(c, 1, pace, kt_desc=True)
                pv_out_block(c, 1, pace, eager_rs=(c == NCH - 1))
                if filler is not None:
                    for _ in filler:
                        pass
    nc.compile()
    return nc


_NC = None
_last_in_maps = None


def _get_nc():
    global _NC
    if _NC is None:
        _NC = _build()
    return _NC


def kernel(x, Wq, Wk, Wv, lambda_q1, lambda_k1, lambda_q2, lambda_k2,
           softmax_scaler, gn_weight):
    x = np.asarray(x, np.float32)
    Wq = np.asarray(Wq, np.float32)
    Wk = np.asarray(Wk, np.float32)
    Wv = np.asarray(Wv, np.float32)
    lam = float(np.exp(np.sum(np.float64(lambda_q1) * np.float64(lambda_k1)))
                - np.exp(np.sum(np.float64(lambda_q2) * np.float64(lambda_k2)))
                + LAMBDA_INIT)
    softmax_scaler = np.asarray(softmax_scaler, np.float32)
    gn_weight = np.asarray(gn_weight, np.float32)
    logp = np.log(np.arange(1, T + 1, dtype=np.float64)).astype(np.float32)

    nc = _get_nc()
    in_maps = []
    for core in range(8):
        b, r = divmod(core, 4)
        qheads = [2 * r, 2 * r + 1, 8 + 2 * r, 8 + 2 * r + 1]
        wq_c = np.concatenate([Wq[:, hh * 128:(hh + 1) * 128] for hh in qheads], axis=1)
        wkv_c = np.concatenate([
            Wk[:, r * 128:(r + 1) * 128],
            Wk[:, (4 + r) * 128:(5 + r) * 128],
            Wv[:, r * 256:(r + 1) * 256],
        ], axis=1)
        lsc = (logp.reshape(NT, 128).T.reshape(128, NT, 1)
               * softmax_scaler[qheads].reshape(1, 1, 4)).astype(np.float32)
        wq_p = wq_c.reshape(NDM, 128, 512).transpose(1, 0, 2)
        wkv_p = wkv_c.reshape(NDM, 128, 512).transpose(1, 0, 2)
        xTb = x[b].T.astype(np.float16)               # [DM, T]
        xP = (xTb.reshape(NDM // 4, 4, 128, NCH, TC)  # [g, j, p, c, t]
              .transpose(3, 0, 2, 1, 4))              # [c, g, p, j, t]
        in_maps.append({
            "xT": np.ascontiguousarray(xP),
            "wq": np.ascontiguousarray(wq_p).astype(np.float16),
            "wkv": np.ascontiguousarray(wkv_p).astype(np.float16),
            "lsc": np.ascontiguousarray(lsc),
            "gnw": np.ascontiguousarray(
                np.broadcast_to(gn_weight[2 * r:2 * r + 2].reshape(1, 2, 256), (128, 2, 256))),
            "neglam": np.full((128, 1), -lam, np.float32),
        })
    global _last_in_maps
    _last_in_maps = in_maps
    res = run_bass_kernel_spmd(nc, in_maps, list(range(8)))
    out = np.empty((B, T, 8, 256), np.float32)
    for core in range(8):
        b, r = divmod(core, 4)
        out[b, :, 2 * r:2 * r + 2, :] = res.results[core]["y"].astype(np.float32)
    return out
